# revision 46
# baseline (speedup 1.0000x reference)
"""TRN2 Bass kernel for nn_CIE_48052094108098 (sparse_attention).

Model (S=2048, B=4, D=512, H=8 -> HH=4 heads/module, DH=128):
  gates = sigmoid(MLP([mean(x[:1024]), mean(x[1024:]), |diff|]))   (per batch)
  xn = LayerNorm(x)
  homo-MHA: same-half block attention, v gated by gates[:,0]
  het-MHA:  cross-half block attention, v gated by gates[:,1]
  y = x + homo + het;  out = y + FFN(y)   (GELU exact)

Sharding: 8 cores = (batch b in 0..3) x (query half p in 0..1). Each core
computes the full output for its (b, half-p) rows; the block masks make each
attention dense over 1024-wide blocks. Zero cross-core communication; the
host reorders columns per core so the kernel is uniform SPMD.

v4 (on top of v3's fp8 DoubleRow pipeline):
- DMA rework: ~11 large descriptors instead of 50 small ones (one dma_start
  is split across all 16 SDMA engines by HW); issue split across the two
  HWDGE rings (sync + scalar) in criticality order. Output flushed as one
  contiguous [128, ND*512] descriptor per s-chunk.
- ACT diet: gater means via DVE tensor_reduce, q/k bias-adds on DVE,
  q-half squares on DVE, y8 casts on ACT only where ACT is idle. ACT keeps
  exp (8/head) + GELU.
- PE warm: junk matmuls sized to the HAM 3.4us window and placed in-queue
  so the LN/rowmath stretch can't re-throttle the clock.
- finish path: attn out normalization multiplies DVE-direct from PSUM
  (no aou staging copy) with gpsimd rank-1 den broadcasts for heads 0-2.
- optional DVE softmax-exp offload (Schraudolph bit-trick) for 3 of 8
  kv-tiles per head to unload ACT (the attention-phase bottleneck).
"""
import sys

sys.path.insert(0, "/opt/trn_rl_repo")

import numpy as np

S, B, D = 2048, 4, 512
SH = S // 2          # 1024 (half)
HH, DH = 4, 128      # heads per module, head dim
HID = 128            # gater hidden
FF = 4 * D           # 2048
NCORE = 8
ND = D // 128        # 4 feature tiles
NFF = FF // 128      # 16
NT = SH // 128       # 8 t-tiles per kv half
NSQ = SH // 512      # 2 query s-chunks
NCONST = 53          # merged const columns (incl -g_b2 at [0:1, 51:53])

# DVE softmax-exp offload: which t-tiles (of 8 per head) use the int8
# Schraudolph bit-trick (one tensor_scalar writing fp8e4m3 bits directly)
# on the vector engine instead of ACT exp.
SCH_TILES = (1, 3, 5, 7)
SCH8_A = float(8 * np.log2(np.e))      # x INV_SQRT_DH at the use site
SCH8_B = float(56 - 0.375)

_CACHED = {}


def build_nc():
    import concourse.mybir as mybir
    import concourse.tile as tile
    from concourse import bacc

    F32 = mybir.dt.float32
    F32R = mybir.dt.float32r
    I32 = mybir.dt.int32
    I8 = mybir.dt.int8
    BF16 = mybir.dt.bfloat16
    F8 = mybir.dt.float8e4
    ACTF = mybir.ActivationFunctionType
    ALU = mybir.AluOpType
    DR = mybir.MatmulPerfMode.DoubleRow

    nc = bacc.Bacc("TRN2", target_bir_lowering=False, debug=False,
                   num_devices=NCORE)

    # ---- DRAM parameters (all partition-major: [128, ...]) ----
    dp = nc.declare_dram_parameter
    x8_d = dp("x8", [128, ND * SH], F8, isOutput=False)      # q half only
    x16_d = dp("x16", [128, 2 * ND * SH], BF16, isOutput=False)  # [half][d][t]
    gw1T_d = dp("gw1T", [128, 12 * HID], BF16, isOutput=False)
    cst_d = dp("cst", [128, NCONST], F32, isOutput=False)
    wqkv_d = {m: dp(f"wqkv_{m}", [128, 3 * ND * D], F8, isOutput=False)
              for m in ("h", "t")}
    wo_d = {m: dp(f"wo_{m}", [128, ND * D], F8, isOutput=False)
            for m in ("h", "t")}
    w1_d = dp("w1", [128, ND * FF], F8, isOutput=False)
    w2_d = dp("w2", [128, NFF * D], F8, isOutput=False)
    zT_d = dp("zT", [128, NSQ * ND * 512], F32, isOutput=True)  # [sq][ot][512]
    import os
    DBG = bool(os.environ.get("K_DBG"))
    if DBG:
        dbg16_d = dp("dbg16", [128, 4 * 1024], BF16, isOutput=True)
        dbg8_d = dp("dbg8", [128, 16 * 1024], F8, isOutput=True)
        dbg32_d = dp("dbg32", [128, 4 * 1024], F32, isOutput=True)

    INV_SQRT_DH = float(1.0 / np.sqrt(DH))

    lp = nc.allow_low_precision("fp8/f32r intermediates; 2e-2 rel-err budget")
    lp.__enter__()
    with tile.TileContext(nc, pool_alloc_mode="queue") as tc:
        const = tc.alloc_tile_pool(name="const", bufs=1)
        big = tc.alloc_tile_pool(name="big", bufs=1)
        ppt = tc.alloc_tile_pool(name="ppt", bufs=4)
        pp8 = tc.alloc_tile_pool(name="pp8", bufs=2)
        pbc = tc.alloc_tile_pool(name="pbc", bufs=2)
        psmall = tc.alloc_tile_pool(name="psmall", bufs=16)
        pz = tc.alloc_tile_pool(name="pz", bufs=1)
        psum = tc.alloc_tile_pool(name="psum", bufs=2, space="PSUM")

        def ps_big2(nm):       # [128,1024] two-bank psum ring
            return psum.tile([128, 1024], F32, name=nm, tag="big2", bufs=2)

        def ps_av2(nm, shape=(128, 1024)):
            return psum.tile(list(shape), F32, name=nm, tag="av2", bufs=2)

        # ---------- on-device constants (no DMA descriptors burned) ------
        ones8_t = const.tile([128, 2, 256], F8)
        nc.vector.memset(ones8_t, 1.0)
        ones1 = ones8_t[:, 0, 0:1]          # [128,1] fp8 for junk lhsT
        onesM = ones8_t[:, :, 0:128]        # [128,2,128] DR stationary:
        eps_t = const.tile([128, 1], F32)   # reductions land PRE-BROADCAST
        nc.vector.memset(eps_t, 1e-5)
        ones16_t = const.tile([128, 128], BF16)
        nc.vector.memset(ones16_t, 1.0)

        # ---------- input DMAs ------------------------------------------
        # sync HWDGE ring carries everything except wqkv_h (scalar ring):
        # queueing DMAs on the ACT ring blocks ACT compute behind ring
        # backpressure (HWDGE admits only a few outstanding transfers).
        x8 = big.tile([128, ND, SH], F8, tag="t_x8")
        nc.sync.dma_start(out=x8, in_=x8_d[:, :].rearrange("p (n f) -> p n f", n=ND))
        wqkv8 = {}
        wqkv8["h"] = big.tile([128, 3, ND, D], F8, name="wqkv8_h", tag="t_wqkvh")
        nc.scalar.dma_start(out=wqkv8["h"],
                            in_=wqkv_d["h"][:, :].rearrange("p (v n f) -> p v n f", v=3, n=ND))
        x16 = big.tile([128, 2, ND, SH], BF16, tag="t_x16")
        nc.sync.dma_start(
            out=x16[:, 0], in_=x16_d[:, 0:ND * SH].rearrange("p (n f) -> p n f", n=ND))
        cst_t = const.tile([128, NCONST], F32)
        nc.sync.dma_start(out=cst_t, in_=cst_d[:, :])
        nc.sync.dma_start(
            out=x16[:, 1], in_=x16_d[:, ND * SH:].rearrange("p (n f) -> p n f", n=ND))
        wqkv8["t"] = big.tile([128, 3, ND, D], F8, name="wqkv8_t", tag="t_wqkvt")
        nc.sync.dma_start(out=wqkv8["t"],
                          in_=wqkv_d["t"][:, :].rearrange("p (v n f) -> p v n f", v=3, n=ND))
        gw1T_t = const.tile([128, 12, HID], BF16)
        nc.sync.dma_start(out=gw1T_t, in_=gw1T_d[:, :].rearrange("p (n f) -> p n f", n=12))
        wo8 = {}
        for m in ("h", "t"):
            wo8[m] = big.tile([128, ND, D], F8, name=f"wo8_{m}", tag=f"t_wo{m}")
            nc.sync.dma_start(out=wo8[m], in_=wo_d[m][:, :].rearrange("p (n f) -> p n f", n=ND))
        w18 = big.tile([128, ND, FF], F8, tag="t_w1")
        nc.sync.dma_start(out=w18, in_=w1_d[:, :].rearrange("p (n f) -> p n f", n=ND))
        w28 = big.tile([128, NFF, D], F8, tag="t_w2")
        nc.sync.dma_start(out=w28, in_=w2_d[:, :].rearrange("p (n f) -> p n f", n=NFF))

        w8 = {("v", m): wqkv8[m][:, 0] for m in ("h", "t")}
        w8.update({("k", m): wqkv8[m][:, 1] for m in ("h", "t")})
        w8.update({("q", m): wqkv8[m][:, 2] for m in ("h", "t")})

        bqk_t = {"h": cst_t[:, 0:8], "t": cst_t[:, 8:16]}
        wbv_t = {"h": cst_t[:, 16:20], "t": cst_t[:, 20:24]}
        bo_t = cst_t[:, 24:28]
        b1_t = cst_t[:, 28:44]
        b2_t = cst_t[:, 44:48]
        gb1_t = cst_t[:, 48:49]
        gw2T_t = cst_t[:, 49:51]
        ngb2_t = cst_t[0:1, 51:53]

        # ---------- PE warmup: junk matmuls sized to the HAM window -----
        # N=512 fp8 non-DR streams keep the PE busy through dep stalls;
        # a batch of ~10 is ~4.3us cold = one full Activity_SHORT window.
        warm1 = ps_big2("warm1")

        def junk(w, n):
            for i in range(n):
                nc.tensor.matmul(w[0:1, 0:512], ones1, ones8_t[:, :, :],
                                 start=True, stop=True)

        junk(warm1, 10)

        # ---------- LN stats (pre-broadcast: M=128 ones stationary) ------
        xn8 = big.tile([128, 2, ND, SH], F8, tag="t_xn8")
        sq8 = big.tile([128, ND, SH], F8, name="xsq8", tag="t_sh1")

        def stats_mm(dst, src):
            for c in range(2):
                for j in range(2):
                    nc.tensor.matmul(dst[:, c * 512:(c + 1) * 512], onesM,
                                     src[:, 2 * j:2 * j + 2, c * 512:(c + 1) * 512],
                                     start=(j == 0), stop=(j == 1), perf_mode=DR)

        sum_q = ps_av2("sum_q")
        stats_mm(sum_q, x8)
        for d in range(ND):            # squares q: split ACT / DVE
            if d < 2:
                nc.scalar.activation(sq8[:, d, :], x8[:, d, :], ACTF.Square)
            else:
                nc.vector.tensor_mul(sq8[:, d, :], x8[:, d, :], x8[:, d, :])
        ssq_q = ps_av2("ssq_q")
        stats_mm(ssq_q, sq8)
        warm2 = ps_big2("warm2")
        junk(warm2, 6)

        # row math on [128, w] pre-broadcast stats: every op uses all 128
        # DVE lanes and the results land in SBUF already broadcast.
        def row_math_B(sum_ps, ssq_ps, rdst, mdst, tag, chunks=2):
            w = 1024 // chunks
            for c in range(chunks):
                s = slice(c * w, (c + 1) * w)
                meanB = ppt.tile([128, w], F32, name=f"meanB{tag}{c}", tag="rmB", bufs=4)
                nc.vector.tensor_scalar_mul(meanB, sum_ps[:, s], 1.0 / D)
                msqB = ppt.tile([128, w], F32, name=f"msqB{tag}{c}", tag="rmB", bufs=4)
                nc.vector.tensor_mul(msqB, meanB, meanB)
                varB = ppt.tile([128, w], F32, name=f"varB{tag}{c}", tag="rmB", bufs=4)
                nc.vector.scalar_tensor_tensor(out=varB, in0=ssq_ps[:, s],
                                               scalar=1.0 / D, in1=msqB,
                                               op0=ALU.mult, op1=ALU.subtract)
                rvarB = ppt.tile([128, w], F32, name=f"rvarB{tag}{c}", tag="rmB", bufs=4)
                nc.vector.reciprocal_approx_fast(rvarB, varB)
                nc.scalar.activation(rdst[:, s], rvarB, ACTF.Sqrt)
                nc.vector.tensor_mul(mdst[:, s], meanB, rdst[:, s])

        rstdB_qs = big.tile([128, 1024], BF16, name="rstdB_qs", tag="t_rq")
        mrB_qs = big.tile([128, 1024], BF16, name="mrB_qs", tag="t_mq")
        row_math_B(sum_q, ssq_q, rstdB_qs, mrB_qs, "q")
        # squares o from x16 on gpsimd (bf16 in/out)
        sq16o = big.tile([128, ND, SH], BF16, name="sq16o", tag="t_sqo")
        for d in range(ND):
            nc.gpsimd.tensor_mul(sq16o[:, d, :], x16[:, 1, d, :], x16[:, 1, d, :])
        junk(warm2, 16)

        # apply q half in bf16 (DVE d0-1, gpsimd d2-3),
        # chunked at 512 so head-0 projections start early
        for c in range(2):
            for d in range(ND):
                eng = nc.vector if d < 2 else nc.gpsimd
                t1 = ppt.tile([128, 512], BF16, name=f"lnt0_{d}{c}", tag="pth",
                              bufs=2)
                eng.tensor_mul(t1, x16[:, 0, d, c * 512:(c + 1) * 512],
                               rstdB_qs[:, c * 512:(c + 1) * 512])
                eng.tensor_sub(xn8[:, 0, d, c * 512:(c + 1) * 512], t1,
                               mrB_qs[:, c * 512:(c + 1) * 512])
        # ---------- gater (1/SH folded into gw1T host-side) -----
        gates = {}
        g128s = {}
        bo_eff_box = {}
        g_in = []

        fsum = {}

        def gater_means():
            # token sums via ACT accum (startup ACT has slack; DVE is the
            # startup critical path). 1/SH folded into gw1T host-side.
            # bf16 casts + |diff| deferred to gater_tail (off the DVE
            # critical queue).
            for half, nm in ((0, "fq"), (1, "fo")):
                for d in range(ND):
                    src_ap = x8[:, d, :] if half == 0 else x16[:, 1, d, :]
                    jk = ppt.tile([128, 1024], F32, name=f"gj{nm}{d}", tag="pt", bufs=1)
                    f = psmall.tile([128, 1], F32, name=f"{nm}{d}", tag="gsm")
                    nc.scalar.activation(jk, src_ap, ACTF.Copy, accum_out=f)
                    fsum[nm, d] = f

        gater_means()
        warm3 = ps_big2("warm3")
        for i in range(10):
            nc.tensor.matmul(warm3[0:1, 0:512], ones1,
                             xn8[:, 0, 0, 0:512], start=True, stop=True)
        rstdB_o = big.tile([128, 1024], BF16, name="rstdB_o", tag="t_ro")
        mrB_o = big.tile([128, 1024], BF16, name="mrB_o", tag="t_mo")

        def stats16_mm(dst, src):
            for c in range(2):
                for d in range(ND):
                    nc.tensor.matmul(dst[:, c * 512:(c + 1) * 512], ones16_t,
                                     src[:, d, c * 512:(c + 1) * 512],
                                     start=(d == 0), stop=(d == ND - 1))

        def stats_o():         # at h0: o-half stats + row math (pre-bcast)
            sum_o = ps_big2("sum_oB")
            stats16_mm(sum_o, x16[:, 1])
            ssq_o = ps_big2("ssq_oB")
            stats16_mm(ssq_o, sq16o)
            row_math_B(sum_o, ssq_o, rstdB_o, mrB_o, "o")

        bo1_box = {}

        def gater_tail():
            fq_l, fo_l = [], []
            for nm, lst in (("fq", fq_l), ("fo", fo_l)):
                for d in range(ND):
                    fb = psmall.tile([128, 1], BF16, name=f"{nm}b{d}", tag="gsm")
                    nc.vector.tensor_copy(fb, fsum[nm, d])
                    lst.append(fb)
            for d in range(ND):
                ad = psmall.tile([128, 1], BF16, name=f"ad{d}", tag="gsm")
                nc.vector.tensor_sub(ad, fq_l[d], fo_l[d])
                ab = psmall.tile([128, 1], BF16, name=f"ab{d}", tag="gsm")
                nc.vector.scalar_tensor_tensor(out=ab, in0=ad, scalar=-1.0,
                                               in1=ad, op0=ALU.mult, op1=ALU.max)
                g_in.append(ab)
            g_in[:0] = fq_l + fo_l      # order: f_q tiles, f_o tiles, |diff|
            g1_psum = psum.tile([128, 1], F32, name="g1_psum", tag="big2", bufs=2)
            for i in range(12):
                nc.tensor.matmul(g1_psum, gw1T_t[:, i, :], g_in[i],
                                 start=(i == 0), stop=(i == 11))
            relu_t = psmall.tile([128, 1], F32, tag="gsm")
            nc.vector.tensor_scalar(out=relu_t, in0=g1_psum, scalar1=gb1_t,
                                    scalar2=0.0, op0=ALU.add, op1=ALU.max)
            for j, m in enumerate(("h", "t")):
                g2_psum = psum.tile([1, 1], F32, name=f"g2_psum{j}", tag="big2", bufs=2)
                nc.tensor.matmul(g2_psum, gw2T_t[:, j:j + 1], relu_t, start=True, stop=True)
                # sigmoid(z+gb2) = 1/(1+exp(-z-gb2)) -- stays on the exp table
                eg = psmall.tile([1, 1], F32, name=f"eg{j}", tag="gsm")
                nc.scalar.activation(eg, g2_psum, ACTF.Exp, scale=-1.0,
                                     bias=ngb2_t[:, j:j + 1])
                ep1 = psmall.tile([1, 1], F32, name=f"ep1{j}", tag="gsm")
                nc.vector.tensor_scalar_add(ep1, eg, 1.0)
                gate = psmall.tile([1, 1], F32, name=f"gate{j}", tag="gsm")
                nc.vector.reciprocal_approx_fast(gate, ep1)
                gates[m] = gate
                g128 = pbc.tile([128, 1], F32, name=f"g128_{j}", tag="g128", bufs=2)
                nc.gpsimd.partition_broadcast(g128, gate)
                g128s[m] = g128
                if m == "h":
                    bo1 = psmall.tile([128, ND], F32, name="bo1", tag="boe", bufs=2)
                    nc.vector.scalar_tensor_tensor(out=bo1, in0=wbv_t["h"], scalar=g128,
                                                   in1=bo_t, op0=ALU.mult, op1=ALU.add)
                    bo1_box["v"] = bo1
                else:
                    bo_eff = psmall.tile([128, ND], F32, name="bo_eff", tag="boe", bufs=2)
                    nc.vector.scalar_tensor_tensor(out=bo_eff, in0=wbv_t["t"], scalar=g128,
                                                   in1=bo1_box["v"], op0=ALU.mult, op1=ALU.add)
                    bo_eff_box["v"] = bo_eff

        def mid_h1():          # at h1: o-half LN apply, entirely on gpsimd
            for d in range(ND):
                t1 = ppt.tile([128, 1024], BF16, name=f"lnt1_{d}", tag="pt2", bufs=2)
                nc.gpsimd.tensor_mul(t1, x16[:, 1, d, :], rstdB_o)
                nc.gpsimd.tensor_sub(xn8[:, 1, d, :], t1, mrB_o)

        # ---------- per-module: V/K/Q proj (fp8 DR), per-head attention --
        ao8 = {}
        ao8["h"] = big.tile([128, HH, SH], F8, name="ao8_h", tag="t_aoh")

        def run_module(m, prelude=None, hooks={}):
            kv = 0 if m == "h" else 1
            wqm, wkm, wvm = w8["q", m], w8["k", m], w8["v", m]
            qt = big.tile([128, HH, SH], F8, name=f"qt_{m}", tag=f"t_qt{m}")
            kt = big.tile([128, HH, SH], F8, name=f"kt_{m}", tag=f"t_kt{m}")
            v8 = big.tile([128, NT, D], F8, name=f"v_{m}", tag=f"t_v{m}")
            aom = ao8[m]

            def kqproj(w, dst, ft, boff, src):
                kp = ps_big2(f"kqp_{m}{boff}{ft}")
                for c in range(NSQ):
                    for j in range(2):
                        nc.tensor.matmul(kp[:, c * 512:(c + 1) * 512],
                                         w[:, 2 * j:2 * j + 2, ft * 128:(ft + 1) * 128],
                                         xn8[:, src, 2 * j:2 * j + 2, c * 512:(c + 1) * 512],
                                         start=(j == 0), stop=(j == 1), perf_mode=DR)
                if boff == 0:   # Q keeps its bias (DVE; ACT is exp-bound)
                    nc.vector.tensor_scalar_add(dst[:, ft, :], kp,
                                                bqk_t[m][:, boff + ft:boff + ft + 1])
                else:           # K bias adds only t-independent score terms,
                    # which cancel in softmax -> plain cast, on ACT
                    nc.scalar.activation(dst[:, ft, :], kp, ACTF.Copy)

            def vproj2(tt):    # two t-tiles into one [128,1024] psum, one copy
                vp = ps_big2(f"vp_{m}{tt}")
                for i in range(2):
                    for j in range(2):
                        nc.tensor.matmul(
                            vp[:, i * 512:(i + 1) * 512],
                            xn8[:, kv, 2 * j:2 * j + 2, (tt + i) * 128:(tt + i + 1) * 128],
                            wvm[:, 2 * j:2 * j + 2, :],
                            start=(j == 0), stop=(j == 1), perf_mode=DR)
                nc.vector.tensor_copy(v8[:, tt:tt + 2, :], vp)

            def avden(av_ps, den_ps, p8, h, u):
                s4 = 2 * u

                def av_mm():
                    for sq in range(NSQ):
                        nc.tensor.matmul(av_ps[:, sq * 512:(sq + 1) * 512],
                                         v8[:, 2 * u:2 * u + 2, h * 128:(h + 1) * 128],
                                         p8[:, s4:s4 + 2, sq * 512:(sq + 1) * 512],
                                         start=(u == 0), stop=(u == NT // 2 - 1),
                                         perf_mode=DR)

                def den_mm():
                    # M=128 ones stationary: den lands PRE-BROADCAST across
                    # partitions (PE cost is N cycles, independent of M)
                    for sq in range(NSQ):
                        nc.tensor.matmul(den_ps[:, sq * 512:(sq + 1) * 512],
                                         ones8_t[:, :, 0:128],
                                         p8[:, s4:s4 + 2, sq * 512:(sq + 1) * 512],
                                         start=(u == 0), stop=(u == NT // 2 - 1),
                                         perf_mode=DR)

                # last pair: finish den first so the 1/den chain starts sooner
                if u == NT // 2 - 1:
                    den_mm()
                    av_mm()
                else:
                    av_mm()
                    den_mm()

            if m == "h":
                vproj2(0)
                kqproj(wkm, kt, 0, 4, kv)
                kqproj(wqm, qt, 0, 0, 0)
                vproj2(2)
                vproj2(4)
                vproj2(6)
            else:
                kqproj(wkm, kt, 0, 4, kv)
                kqproj(wqm, qt, 0, 0, 0)
                for tt in range(0, NT, 2):
                    vproj2(tt)
            if prelude is not None:
                prelude()

            def finish(h, av_ps, recipB):
                # aom = (av * gate) * (1/den): one DVE pass, PSUM-direct
                nc.vector.scalar_tensor_tensor(
                    out=aom[:, h, :], in0=av_ps, scalar=g128s[m],
                    in1=recipB, op0=ALU.mult, op1=ALU.mult)

            pend = None
            for h in range(HH):
                if pend is not None:
                    finish(*pend)
                    pend = None
                if h in hooks:
                    hooks[h]()
                av_ps = ps_av2(f"av_{m}{h}")
                den_ps = ps_av2(f"den_{m}{h}")
                p8 = pp8.tile([128, NT, SH], F8, name=f"p8_{m}{h}", tag="p8", bufs=2)
                for u in range(NT // 2):
                    for tt in (2 * u, 2 * u + 1):
                        sp = ps_big2(f"sp_{m}{h}{tt}")
                        for sq in range(NSQ):
                            nc.tensor.matmul(sp[:, sq * 512:(sq + 1) * 512],
                                             kt[:, h, tt * 128:(tt + 1) * 128],
                                             qt[:, h, sq * 512:(sq + 1) * 512],
                                             start=True, stop=True)
                        if tt in SCH_TILES:
                            # int8 Schraudolph exp on DVE: (a*s + b) rounded
                            # to int8 IS the fp8e4m3 bit pattern of ~exp(s)
                            nc.vector.tensor_scalar(
                                out=p8[:, tt, :].bitcast(I8),
                                in0=sp, scalar1=SCH8_A * INV_SQRT_DH,
                                scalar2=SCH8_B, op0=ALU.mult, op1=ALU.add)
                        else:
                            nc.scalar.activation(p8[:, tt, :], sp, ACTF.Exp,
                                                 scale=INV_SQRT_DH)
                    if u > 0:
                        avden(av_ps, den_ps, p8, h, u - 1)
                        if u == 2 and h + 1 < HH:   # prefetch next head's K/Q
                            kqproj(wkm, kt, h + 1, 4, kv)
                            kqproj(wqm, qt, h + 1, 0, 0)
                avden(av_ps, den_ps, p8, h, NT // 2 - 1)
                recipB = pbc.tile([128, 1024], F32, name=f"rcp_{m}{h}", tag="bc", bufs=2)
                nc.vector.reciprocal_approx_fast(recipB, den_ps)
                pend = (h, av_ps, recipB)
            return lambda: finish(*pend)

        def h0_hook():
            stats_o()
            gater_tail()

        fin_h = run_module("h", hooks={0: h0_hook, 1: mid_h1})
        ao8["t"] = big.tile([128, HH, SH], F8, name="ao8_t", tag="t_aot")
        fin_t = run_module("t", prelude=fin_h)
        fin_t()

        # ---------- out-proj (fp8 DR, both modules into one psum) --------
        y32 = big.tile([128, ND, SH], F32, tag="t_y32")
        y8 = big.tile([128, ND, SH], F8, name="y8", tag="t_sh1")

        def op_partial(ft):
            op = ps_big2(f"op_{ft}")
            for mi, m in enumerate(("h", "t")):
                for j in range(2):
                    if mi == 1 and j == 1:
                        continue        # needs the last head: deferred
                    for sq in range(NSQ):
                        nc.tensor.matmul(op[:, sq * 512:(sq + 1) * 512],
                                         wo8[m][:, 2 * j:2 * j + 2, ft * 128:(ft + 1) * 128],
                                         ao8[m][:, 2 * j:2 * j + 2, sq * 512:(sq + 1) * 512],
                                         start=(mi == 0 and j == 0), stop=False,
                                         perf_mode=DR)
            return op

        def op_tail(ft, op):
            for sq in range(NSQ):
                nc.tensor.matmul(op[:, sq * 512:(sq + 1) * 512],
                                 wo8["t"][:, 2:4, ft * 128:(ft + 1) * 128],
                                 ao8["t"][:, 2:4, sq * 512:(sq + 1) * 512],
                                 start=False, stop=True, perf_mode=DR)
            nc.vector.scalar_tensor_tensor(
                out=y32[:, ft, :], in0=op, scalar=bo_eff_box["v"][:, ft:ft + 1],
                in1=x16[:, 0, ft, :], op0=ALU.add, op1=ALU.add)
            # fp8 cast for FFN input on ACT (idle between exp and GELU phases)
            nc.scalar.activation(y8[:, ft, :], y32[:, ft, :], ACTF.Copy)

        op0 = op_partial(0)
        op1 = op_partial(1)
        op_tail(0, op0)
        op2 = op_partial(2)
        op_tail(1, op1)
        op3 = op_partial(3)
        op_tail(2, op2)
        op_tail(3, op3)

        if DBG:
            nc.sync.dma_start(out=dbg16_d[:, 0:1024], in_=rstdB_qs)
            nc.sync.dma_start(out=dbg16_d[:, 1024:2048], in_=mrB_qs)
            nc.sync.dma_start(out=dbg16_d[:, 2048:3072], in_=rstdB_o)
            nc.sync.dma_start(out=dbg16_d[:, 3072:4096], in_=mrB_o)
            nc.sync.dma_start(out=dbg8_d[:, 0:8192],
                              in_=xn8[:, :, :, :].rearrange("p a n f -> p (a n f)"))
            nc.sync.dma_start(out=dbg8_d[:, 8192:12288],
                              in_=ao8["h"][:, :, :].rearrange("p n f -> p (n f)"))
            nc.sync.dma_start(out=dbg8_d[:, 12288:16384],
                              in_=ao8["t"][:, :, :].rearrange("p n f -> p (n f)"))
            nc.sync.dma_start(out=dbg32_d[:, :],
                              in_=y32[:, :, :].rearrange("p n f -> p (n f)"))

        # ---------- FFN (fp8 DR both layers) ----------
        for sq in range(NSQ):
            z_ps = [ps_av2(f"z2_{sq}{i}") for i in range(2)]
            h8_t = {}

            def ffn1(ff, sq=sq, h8_t=h8_t):
                hp = ps_big2(f"hp_{sq}{ff}")
                for j in range(2):
                    nc.tensor.matmul(hp[:, 0:512],
                                     w18[:, 2 * j:2 * j + 2, ff * 128:(ff + 1) * 128],
                                     y8[:, 2 * j:2 * j + 2, sq * 512:(sq + 1) * 512],
                                     start=(j == 0), stop=(j == 1), perf_mode=DR)
                w = ff // 2
                if ff % 2 == 0:
                    h8_t[w] = pp8.tile([128, 2, 512], F8, name=f"h8_{sq}{w}",
                                       tag="h8", bufs=3)
                nc.scalar.activation(h8_t[w][:, ff % 2, :], hp[:, 0:512], ACTF.Gelu,
                                     bias=b1_t[:, ff:ff + 1])

            def ffn2(w, sq=sq, h8_t=h8_t, z_ps=z_ps):
                for ot in range(ND):
                    nc.tensor.matmul(z_ps[ot // 2][:, (ot % 2) * 512:(ot % 2 + 1) * 512],
                                     w28[:, 2 * w:2 * w + 2, ot * 128:(ot + 1) * 128],
                                     h8_t[w], start=(w == 0), stop=(w == NFF // 2 - 1),
                                     perf_mode=DR)

            for w in range(NFF // 2):
                ffn1(2 * w)
                ffn1(2 * w + 1)
                if w > 1:
                    ffn2(w - 2)
            ffn2(NFF // 2 - 2)
            ffn2(NFF // 2 - 1)
            z_t = pz.tile([128, ND, 512], F32, name=f"z_t{sq}", tag="z")
            for ot in range(ND):
                nc.vector.scalar_tensor_tensor(
                    out=z_t[:, ot, :], in0=z_ps[ot // 2][:, (ot % 2) * 512:(ot % 2 + 1) * 512],
                    scalar=b2_t[:, ot:ot + 1],
                    in1=y32[:, ot, sq * 512:(sq + 1) * 512],
                    op0=ALU.add, op1=ALU.add)
                if ot % 2 == 1:   # flush per ot-pair, alternating rings
                    base = sq * ND * 512 + (ot - 1) * 512
                    eng = nc.sync if ot == 1 else nc.scalar
                    eng.dma_start(
                        out=zT_d[:, base:base + 1024],
                        in_=z_t[:, ot - 1:ot + 1, :].rearrange("p n f -> p (n f)"))

        psum.release()
        pz.release()
        psmall.release()
        pbc.release()
        pp8.release()
        ppt.release()
        big.release()
        const.release()

    lp.__exit__(None, None, None)
    nc.finalize()
    return nc


def _pm(a, n):
    """[n*128, cols] -> partition-major [128, n*cols]."""
    a = np.ascontiguousarray(a)
    return np.ascontiguousarray(
        a.reshape(n, 128, -1).transpose(1, 0, 2).reshape(128, -1))


def _prep_inputs(sequence, g_w1, g_b1, g_w2, g_b2, ln_g, ln_b,
                 homo_in_w, homo_in_b, homo_out_w, homo_out_b,
                 het_in_w, het_in_b, het_out_w, het_out_b,
                 ffn_w1, ffn_b1, ffn_w2, ffn_b2):
    import ml_dtypes
    f8 = ml_dtypes.float8_e4m3
    f32 = np.float32
    cc = np.ascontiguousarray

    def rev(v, n):     # [n*128] bias -> [128, n] column layout
        return np.asarray(v, f32).reshape(n, 128).T

    shared = {}
    ln_g = np.asarray(ln_g, f32)
    ln_b = np.asarray(ln_b, f32)
    cst = np.zeros((128, NCONST), f32)
    for mi, (m, in_w, in_b, out_w) in enumerate(
            (("h", homo_in_w, homo_in_b, homo_out_w),
             ("t", het_in_w, het_in_b, het_out_w))):
        in_w = np.asarray(in_w, f32)
        in_b = np.asarray(in_b, f32)
        out_w = np.asarray(out_w, f32)
        wq, wk, wv = in_w[0:D], in_w[D:2 * D], in_w[2 * D:3 * D]
        # fold LN affine into the projections: W' = W*diag(g), b' = b + W@ln_b
        # pack [v|k|q] into one tensor for a single DMA
        shared[f"wqkv_{m}"] = cc(np.concatenate(
            [_pm((wv * ln_g).T.astype(f8), ND),
             _pm((wk * ln_g).T.astype(f8), ND),
             _pm((wq * ln_g).T.astype(f8), ND)], axis=1))
        bqk = in_b[0:2 * D].copy()
        bqk[0:D] += wq @ ln_b
        bqk[D:2 * D] += wk @ ln_b
        cst[:, 8 * mi:8 * mi + 8] = rev(bqk, 8)
        cst[:, 16 + 4 * mi:20 + 4 * mi] = rev(out_w @ (in_b[2 * D:3 * D] + wv @ ln_b), ND)
        shared[f"wo_{m}"] = _pm(out_w.T.astype(f8), ND)
    cst[:, 24:28] = rev(np.asarray(homo_out_b, f32) + np.asarray(het_out_b, f32), ND)
    cst[:, 28:44] = rev(ffn_b1, NFF)
    cst[:, 44:48] = rev(ffn_b2, ND)
    cst[:, 48] = np.asarray(g_b1, f32)
    cst[:, 49:51] = np.asarray(g_w2, f32).T
    cst[0, 51:53] = -np.asarray(g_b2, f32)
    shared["cst"] = cc(cst)
    shared["w1"] = _pm(np.asarray(ffn_w1, f32).T.astype(f8), ND)
    shared["w2"] = _pm(np.asarray(ffn_w2, f32).T.astype(f8), NFF)

    # token-mean folded into gw1T (device sends raw token sums)
    bf16 = ml_dtypes.bfloat16
    g_w1 = np.asarray(g_w1, f32)
    gw1T = (g_w1.T / SH).astype(bf16)                # [1536, HID]: [f_s|f_b|diff]
    gw1T_swap = np.concatenate([gw1T[D:2 * D], gw1T[0:D], gw1T[2 * D:]], axis=0)

    seq = np.asarray(sequence, f32)
    in_maps = []
    for core in range(NCORE):
        b, p = core // 2, core % 2
        xb = seq[:, b, :]                            # [S, D]
        xq = xb[p * SH:(p + 1) * SH]
        xo = xb[(1 - p) * SH:(2 - p) * SH]
        mm = dict(shared)
        mm["x8"] = _pm(xq.T.astype(f8), ND)
        mm["x16"] = cc(np.concatenate(
            [_pm(xq.T.astype(bf16), ND), _pm(xo.T.astype(bf16), ND)], axis=1))
        mm["gw1T"] = _pm(gw1T if p == 0 else gw1T_swap, 12)
        in_maps.append(mm)
    return in_maps


def kernel(**inputs):
    from concourse.bass_utils import run_bass_kernel_spmd

    if "nc" not in _CACHED:
        _CACHED["nc"] = build_nc()
    nc = _CACHED["nc"]

    in_maps = _prep_inputs(**{k: np.asarray(v) for k, v in inputs.items()})
    core_ids = list(range(NCORE))
    res = run_bass_kernel_spmd(nc, in_maps, core_ids)

    out = np.empty((S, B, D), np.float32)
    for core in range(NCORE):
        b, p = core // 2, core % 2
        z = res.results[core]["zT"].reshape(128, NSQ, ND, 512)
        for sq in range(NSQ):
            blk = z[:, sq].transpose(1, 0, 2).reshape(D, 512).T  # [512, D]
            out[p * SH + sq * 512:p * SH + (sq + 1) * 512, b, :] = blk
    return out
